# revision 19
# baseline (speedup 1.0000x reference)
"""Trainium2 Bass kernel for the SE-attention block.

Math (per batch b):
    s[n]   = sum_c x[b,c,n]
    att[c] = sum_n x[b,c,n] * s[n]
    h      = relu(bn(W1 @ att))          (BN folded into scale/bias on host)
    a      = sigmoid(W2 @ h)
    out    = x[b] * a[:, None]

Sharding: data-parallel over batch B=16 across 8 cores (2 batches/core).
Weights replicated. No collectives.

Per-core plan (natural layout, x tiles [128, 4096] per channel-block,
processed in two n-halves of 2048):
  - DVE+POOL: tree-sum the 4 channel tiles (tA=x0+x1 on DVE, tB=x2+x3 on
         GpSimd, tA+=tB on DVE) so only ONE tile goes through the PE's
         expensive 2-pass fp32 stream.
  - PE:  s[1,n] = colsum(tA) via ones[128,1] (M=1), then broadcast to all
         128 partitions via K=1 matmul with ones[1,128] weights.
  - DVE: fused scalar_tensor_tensor: attp[128,1] = rowsum(x_tile * sB).
  - PE:  tiny MLP matmuls (W1T/W2T pre-transposed on host), both half
         partials folded into the PSUM accumulation.
  - ACT: BN+ReLU / sigmoid on [128,1]; final out = x * a in place over the
         x tile via activation(Copy, scale=a); DMA out straight from it.
"""

import numpy as np

try:
    import concourse.bass as bass
except ImportError:  # fresh grading dir: repo not on sys.path
    import sys

    for p in ("/opt/trn_rl_repo", "/root/.axon_site/_ro/trn_rl_repo"):
        if p not in sys.path:
            sys.path.insert(0, p)
    import concourse.bass as bass

import concourse.tile as tile
from concourse import bacc, mybir
from concourse.bass_utils import run_bass_kernel_spmd

F32 = mybir.dt.float32
AF = mybir.ActivationFunctionType
ALU = mybir.AluOpType

B, C, N = 16, 512, 4096
CR = 128          # squeeze dim C//4
NCORES = 8
BPC = B // NCORES  # batches per core
P = 128
CT = C // P        # channel tiles per batch
NH = N // 2        # psum half width (4 banks)
NCHUNK = 512       # matmul free-dim max (one psum bank)
BN_EPS = 1e-5
MODE = "stt"  # att fusion: "ttr" | "stt" | "split"

_nc_cache = None


def _build():
    nc = bacc.Bacc(None, target_bir_lowering=False)
    x = nc.declare_dram_parameter("x", [BPC, C, N], F32, isOutput=False)
    w1t = nc.declare_dram_parameter("w1t", [C, CR], F32, isOutput=False)
    w2t = nc.declare_dram_parameter("w2t", [CR, C], F32, isOutput=False)
    bns = nc.declare_dram_parameter("bns", [CR, 1], F32, isOutput=False)
    bnb = nc.declare_dram_parameter("bnb", [CR, 1], F32, isOutput=False)
    y = nc.declare_dram_parameter("y", [BPC, C, N], F32, isOutput=True)

    with tile.TileContext(nc) as tc:
        with (
            tc.tile_pool(name="consts", bufs=1) as consts,
            tc.tile_pool(name="x", bufs=2 * CT) as xpool,
            tc.tile_pool(name="big", bufs=3) as big,
            tc.tile_pool(name="small", bufs=4 * CT) as small,
            tc.tile_pool(name="psum", bufs=1, space="PSUM") as psum,
        ):
            ones_col = consts.tile([P, 1], F32)
            nc.vector.memset(ones_col, 1.0)
            ones_row = consts.tile([1, P], F32)
            nc.vector.memset(ones_row, 1.0)
            w1t_sb = consts.tile([P, CT, CR], F32)
            nc.sync.dma_start(
                out=w1t_sb, in_=w1t[:].rearrange("(t p) o -> p t o", p=P)
            )
            w2t_sb = consts.tile([P, C], F32)
            nc.sync.dma_start(out=w2t_sb, in_=w2t[:])
            bns_sb = consts.tile([P, 1], F32)
            nc.sync.dma_start(out=bns_sb, in_=bns[:])
            bnb_sb = consts.tile([P, 1], F32)
            nc.sync.dma_start(out=bnb_sb, in_=bnb[:])

            # Pre-clear const dependencies: make PE/ACT observe each const's
            # semaphore once via tiny dummy consumers, so real instructions
            # carry at most one fresh sync-wait (avoids event-sem splits on
            # the critical path).
            scratch_ps = psum.tile([P, 1], F32, tag="sps", name="scratch_ps")
            nc.tensor.matmul(
                scratch_ps, ones_row, ones_row[:, :1], start=True, stop=True
            )
            nc.tensor.matmul(
                scratch_ps, w1t_sb[:, 0, :], ones_col, start=True, stop=True
            )
            nc.tensor.matmul(
                scratch_ps, w2t_sb[:, :P], ones_col, start=True, stop=True
            )
            scratch_sb = consts.tile([P, 1], F32)
            nc.scalar.copy(scratch_sb, bns_sb)
            nc.scalar.copy(scratch_sb, bnb_sb)

            for b in range(BPC):
                xt = []
                for t in range(CT):
                    xtile = xpool.tile([P, N], F32, tag="x", name=f"x_{b}_{t}")
                    nc.sync.dma_start(out=xtile, in_=x[b, t * P : (t + 1) * P, :])
                    xt.append(xtile)

                attp = [
                    [
                        small.tile([P, 1], F32, tag="attp", name=f"attp_{b}_{h}_{t}")
                        for t in range(CT)
                    ]
                    for h in range(2)
                ]
                for h in range(2):
                    lo, hi = h * NH, (h + 1) * NH
                    # tree-sum the 4 channel tiles down to one [128, NH] tile
                    tA = big.tile([P, NH], F32, tag="tA", bufs=2, name=f"tA_{b}_{h}")
                    tB = big.tile([P, NH], F32, tag="tB", bufs=2, name=f"tB_{b}_{h}")
                    nc.vector.tensor_add(tA, xt[0][:, lo:hi], xt[1][:, lo:hi])
                    nc.gpsimd.tensor_add(tB, xt[2][:, lo:hi], xt[3][:, lo:hi])
                    nc.vector.tensor_add(tA, tA, tB)
                    # s[1, n] = colsum(tA) (M=1: trivial weight load)
                    sps = psum.tile([1, NH], F32, tag="sps", name=f"sps_{b}_{h}")
                    for j in range(NH // NCHUNK):
                        nc.tensor.matmul(
                            sps[:, j * NCHUNK : (j + 1) * NCHUNK],
                            ones_col,
                            tA[:, j * NCHUNK : (j + 1) * NCHUNK],
                            start=True,
                            stop=True,
                        )
                    s_sb = small.tile(
                        [1, NH], F32, tag="srow", bufs=2, name=f"s_{b}_{h}"
                    )
                    nc.scalar.copy(s_sb, sps)
                    # broadcast s across all 128 partitions via K=1 matmul
                    sb = psum.tile([P, NH], F32, tag="sb", name=f"sb_{b}_{h}")
                    for j in range(NH // NCHUNK):
                        nc.tensor.matmul(
                            sb[:, j * NCHUNK : (j + 1) * NCHUNK],
                            ones_row,
                            s_sb[:, j * NCHUNK : (j + 1) * NCHUNK],
                            start=True,
                            stop=True,
                        )
                    for t in range(CT):
                        # reuse the tree buffers for the STT side-output
                        junk = big.tile(
                            [P, NH], F32, tag="tB", bufs=2, name=f"junk_{b}_{h}_{t}"
                        )
                        # fused: junk = (x*1.0)*sb, attp = rowsum(junk)
                        nc.vector.scalar_tensor_tensor(
                            out=junk,
                            in0=xt[t][:, lo:hi],
                            scalar=1.0,
                            in1=sb,
                            op0=ALU.mult,
                            op1=ALU.mult,
                            accum_out=attp[h][t],
                        )

                # h = relu(bn_scale * (W1 @ att) + bn_bias); att = attp0 + attp1
                # folded into the PSUM accumulation (8 rank-updates)
                hpsum = psum.tile([P, 1], F32, tag="sps", name=f"hpsum_{b}")
                for i, (h, t) in enumerate(
                    [(h, t) for h in range(2) for t in range(CT)]
                ):
                    nc.tensor.matmul(
                        hpsum,
                        w1t_sb[:, t, :],
                        attp[h][t],
                        start=(i == 0),
                        stop=(i == 2 * CT - 1),
                    )
                hb = small.tile([P, 1], F32, tag="hb", name=f"hb_{b}")
                nc.scalar.activation(
                    hb, hpsum, AF.Relu, bias=bnb_sb, scale=bns_sb
                )

                # a = sigmoid(W2 @ h), per 128-channel chunk
                avec = []
                for t in range(CT):
                    apsum = psum.tile([P, 1], F32, tag="sps", name=f"apsum_{b}_{t}")
                    nc.tensor.matmul(
                        apsum,
                        w2t_sb[:, t * P : (t + 1) * P],
                        hb,
                        start=True,
                        stop=True,
                    )
                    a_t = small.tile([P, 1], F32, tag="a", name=f"a_{b}_{t}")
                    nc.scalar.activation(a_t, apsum, AF.Sigmoid)
                    avec.append(a_t)

                # out = x * a, scaled in place over the x tile, stored from it
                for t in range(CT):
                    nc.scalar.mul(xt[t], xt[t], avec[t])
                    nc.sync.dma_start(
                        out=y[b, t * P : (t + 1) * P, :], in_=xt[t]
                    )
    return nc


def _get_nc():
    global _nc_cache
    if _nc_cache is None:
        _nc_cache = _build()
        if not _nc_cache.is_finalized():
            _nc_cache.finalize()
    return _nc_cache


def _host_prep(x, W1, gamma, beta, running_mean, running_var, W2):
    x = np.asarray(x, dtype=np.float32)
    rstd = 1.0 / np.sqrt(np.asarray(running_var, np.float32) + BN_EPS)
    bns = (np.asarray(gamma, np.float32) * rstd).reshape(CR, 1)
    bnb = (
        np.asarray(beta, np.float32)
        - np.asarray(running_mean, np.float32) * bns[:, 0]
    ).reshape(CR, 1)
    w1t = np.ascontiguousarray(np.asarray(W1, np.float32).T)  # [C, CR]
    w2t = np.ascontiguousarray(np.asarray(W2, np.float32).T)  # [CR, C]
    in_maps = []
    for c in range(NCORES):
        in_maps.append(
            {
                "x": np.ascontiguousarray(x[c * BPC : (c + 1) * BPC]),
                "w1t": w1t,
                "w2t": w2t,
                "bns": np.ascontiguousarray(bns, np.float32),
                "bnb": np.ascontiguousarray(bnb, np.float32),
            }
        )
    return in_maps


def _run(inputs, **spmd_kwargs):
    in_maps = _host_prep(**inputs)
    res = run_bass_kernel_spmd(
        _get_nc(), in_maps, list(range(NCORES)), **spmd_kwargs
    )
    out = np.concatenate([res.results[c]["y"] for c in range(NCORES)], axis=0)
    return out.astype(np.float32, copy=False), res


def kernel(**inputs):
    out, _ = _run(inputs)
    return out
